# revision 10
# baseline (speedup 1.0000x reference)
"""AttentionNet kernel for Trainium2: 8-core data-parallel over batch.

Reference computation (per batch element b):
  emb    = x.reshape(N,64) @ conv_w + conv_b          [N,512]
  x_real = emb * mask[:,None]
  query  = sum_n(x_real) / (sum(mask)+1e-5)           [512]
  q_proj = query @ Uq                                 [512]
  r_proj = x_real @ Ur                                [N,512]
  logits = tanh(q_proj + r_proj) @ Ua                 [N]
  attn   = softmax(logits masked)                     [N]
  out    = attn @ x_real                              [512]

Kernel restructure (everything per core, batch shard of 256):
  xm = x * mask;  xaT[d, (b,n)] = [xm.T ; mask_row ; indicator_rows]  (69 rows)
  r_proj+q_proj fused:  z = lhsT.T @ xaT  with lhsT = [W@Ur ; b@Ur ; q_proj cols]
  (the 4 indicator rows select the right q_proj column per batch-of-macro)
  logits = Ua_rep.T @ tanh(z)   (Ua replicated across 128 cols -> logits
  replicated across partitions, so exp() broadcast is free)
  unnormalized softmax: e = exp(logits); weighted reduce of [xm;mask] rows by e
  gives both sum_n(e * x_real-preimage) and Z = sum_n(e * mask) in one pass.
  out = (esum.T @ [W;b]) * (1/Z)
  query path: xa_sum (grouped reduce) -> q_proj = (Wa@Uq).T @ xa_sum / denom
"""

import os
import sys

sys.path.insert(0, "/opt/trn_rl_repo")

import numpy as np
from contextlib import ExitStack

import concourse.bass as bass
import concourse.bacc as bacc
import concourse.tile as tile
from concourse import mybir

B, N, DOBJ, DM = 2048, 128, 64, 512
NCORES = 8
BSH = B // NCORES          # 256 batch per core
MB = 4                     # batch elements per macro tile
NMAC = BSH // MB           # 64 macro tiles
R = MB * N                 # 512 rows per macro
KC = 4                     # 512 = 4 chunks of 128 along d_model
F32 = mybir.dt.float32
F32R = mybir.dt.float32r
AF = mybir.ActivationFunctionType
ALU = mybir.AluOpType

MM_FAST = os.environ.get("MM_FP32", "0") != "1"   # float32r for big matmuls
DT_MM_GLOBAL = None


DT_MM = None  # set in build_nc


def build_nc():
    nc = bacc.Bacc("TRN2", target_bir_lowering=False, debug=False, num_devices=1)

    x = nc.dram_tensor("x", [BSH, N * DOBJ], F32, kind="ExternalInput")
    mask = nc.dram_tensor("mask", [BSH, N], F32, kind="ExternalInput")
    w = nc.dram_tensor("conv_w", [DOBJ, DM], F32, kind="ExternalInput")
    cb = nc.dram_tensor("conv_b", [1, DM], F32, kind="ExternalInput")
    uq = nc.dram_tensor("Uq", [DM, DM], F32, kind="ExternalInput")
    ur = nc.dram_tensor("Ur", [DM, DM], F32, kind="ExternalInput")
    ua = nc.dram_tensor("Ua", [1, DM], F32, kind="ExternalInput")
    ind = nc.dram_tensor("ind4", [MB, R], F32, kind="ExternalInput")
    ident = nc.dram_tensor("ident", [128, 128], F32, kind="ExternalInput")
    out = nc.dram_tensor("out", [BSH, DM], F32, kind="ExternalOutput")

    # persistent SBUF tensors
    DT_MM = F32R if MM_FAST else F32
    xaT = nc.alloc_sbuf_tensor("xaT", [69, NMAC * R], DT_MM).ap()      # 131KB/part
    wb = nc.alloc_sbuf_tensor("wb", [65, DM], F32).ap()              # [[W];[b]]
    wura = nc.alloc_sbuf_tensor("wura", [65, DM], F32).ap()          # [[W@Ur];[b@Ur]]
    wauq = nc.alloc_sbuf_tensor("wauq", [65, DM], DT_MM).ap()          # [[W@Uq];[b@Uq]]
    uarep = nc.alloc_sbuf_tensor("uarep", [128, DM], DT_MM).ap()       # Ua chunks bcast
    maskT = nc.alloc_sbuf_tensor("maskT", [128, BSH], F32).ap()      # [n, b]
    mnat = nc.alloc_sbuf_tensor("mnat", [128, BSH], F32).ap()        # [b, n] 2 tiles
    recipd = nc.alloc_sbuf_tensor("recipd", [128, 2], F32).ap()      # 1/denom
    recipz = nc.alloc_sbuf_tensor("recipz", [128, 2], F32).ap()      # 1/Z
    qptt = nc.alloc_sbuf_tensor("qptt", [128, 2 * DM], DT_MM).ap()     # [b, k] halves
    xasum = nc.alloc_sbuf_tensor("xasum", [65, BSH], DT_MM).ap()
    xaesum = nc.alloc_sbuf_tensor("xaesum", [65, BSH], F32).ap()
    rpw = nc.alloc_sbuf_tensor("rpw", [69, 2 * DM], DT_MM).ap()        # lhsT ring x2
    id_sb = nc.alloc_sbuf_tensor("id_sb", [128, 128], F32).ap()
    ua_nat = nc.alloc_sbuf_tensor("ua_nat", [1, DM], F32).ap()
    wbt = nc.alloc_sbuf_tensor("wbt", [128, 4 * 65], F32).ap()       # W.T chunks

    xap = x.ap()
    maskap = mask.ap()

    with tile.TileContext(nc) as tc, ExitStack() as big:
        # ---------------- setup: loads ----------------
        nc.sync.dma_start(out=id_sb, in_=ident.ap())
        nc.sync.dma_start(out=wb[0:64, :], in_=w.ap())
        nc.sync.dma_start(out=wb[64:65, :], in_=cb.ap())
        nc.sync.dma_start(out=ua_nat, in_=ua.ap())
        for h in range(2):
            nc.sync.dma_start(
                out=mnat[:, h * 128:(h + 1) * 128],
                in_=maskap[h * 128:(h + 1) * 128, :],
            )
        # mask row of xaT: the full mask shard, flattened row-major = (b,n)
        nc.sync.dma_start(
            out=xaT[64:65, :],
            in_=bass.AP(tensor=mask, offset=0, ap=[[0, 1], [1, BSH * N]]).bitcast(xaT.dtype),
        )
        # indicator rows, replicated across all macros
        nc.sync.dma_start(
            out=xaT[65:69, :].rearrange("p (m r) -> p m r", r=R),
            in_=bass.AP(tensor=ind, offset=0, ap=[[R, MB], [0, NMAC], [1, R]]).bitcast(xaT.dtype),
        )

        # ---------------- setup: weight precompute ----------------
        with ExitStack() as ctx:
            sps = ctx.enter_context(tc.tile_pool(name="sps", bufs=2, space="PSUM"))
            ssb = ctx.enter_context(tc.tile_pool(name="ssb", bufs=4))

            # W.T chunks: transpose [65,128] slices of [[W];[b]] -> [128,65]
            for mc in range(KC):
                tp = sps.tile([128, 65], F32, tag="tp")
                nc.tensor.transpose(
                    tp, wb[:, mc * 128:(mc + 1) * 128], id_sb[0:65, 0:65]
                )
                nc.vector.tensor_copy(out=wbt[:, mc * 65:(mc + 1) * 65], in_=tp)

            urt = []
            uqt = []
            for mc in range(KC):
                t1 = ssb.tile([128, DM], F32, tag="urt")
                nc.sync.dma_start(out=t1, in_=ur.ap()[mc * 128:(mc + 1) * 128, :])
                urt.append(t1)
                t2 = ssb.tile([128, DM], F32, tag="uqt")
                nc.sync.dma_start(out=t2, in_=uq.ap()[mc * 128:(mc + 1) * 128, :])
                uqt.append(t2)

            # WUra = Wa @ Ur, WaUq = Wa @ Uq  (fp32 accumulating matmuls)
            wura_ps = sps.tile([65, DM], F32, tag="wu")
            for mc in range(KC):
                nc.tensor.matmul(
                    wura_ps, wbt[:, mc * 65:(mc + 1) * 65], urt[mc],
                    start=(mc == 0), stop=(mc == KC - 1),
                )
            nc.vector.tensor_copy(out=wura, in_=wura_ps)
            wauq_ps = sps.tile([65, DM], F32, tag="wu")
            for mc in range(KC):
                nc.tensor.matmul(
                    wauq_ps, wbt[:, mc * 65:(mc + 1) * 65], uqt[mc],
                    start=(mc == 0), stop=(mc == KC - 1),
                )
            nc.vector.tensor_copy(out=wauq, in_=wauq_ps)

            # static rows of the r_proj lhsT ring (both parities)
            for p in range(2):
                nc.scalar.copy(out=rpw[0:65, p * DM:(p + 1) * DM], in_=wura)

            # maskT = mask.T ; denom; msum row into xasum[64]
            for h in range(2):
                mt = sps.tile([128, 128], F32, tag="tp")
                nc.tensor.transpose(mt, mnat[:, h * 128:(h + 1) * 128], id_sb)
                nc.vector.tensor_copy(out=maskT[:, h * 128:(h + 1) * 128], in_=mt)

                dn = ssb.tile([128, 1], F32, tag="dn")
                nc.vector.reduce_sum(
                    out=dn, in_=mnat[:, h * 128:(h + 1) * 128],
                    axis=mybir.AxisListType.X,
                )
                dt_ps = sps.tile([1, 128], F32, tag="tp")
                nc.tensor.transpose(dt_ps, dn, id_sb)
                nc.vector.tensor_copy(
                    out=xasum[64:65, h * 128:(h + 1) * 128], in_=dt_ps
                )
                dn2 = ssb.tile([128, 1], F32, tag="dn2")
                nc.vector.tensor_scalar(
                    out=dn2, in0=dn, scalar1=1e-5, scalar2=None, op0=ALU.add
                )
                nc.vector.reciprocal(out=recipd[:, h:h + 1], in_=dn2)

            # Ua replicated chunks
            for kc in range(KC):
                uac_ps = sps.tile([128, 1], F32, tag="tp")
                nc.tensor.transpose(
                    uac_ps, ua_nat[0:1, kc * 128:(kc + 1) * 128], id_sb[0:1, 0:1]
                )
                uac = ssb.tile([128, 1], F32, tag="uac")
                nc.vector.tensor_copy(out=uac, in_=uac_ps)
                nc.vector.tensor_copy(
                    out=uarep[:, kc * 128:(kc + 1) * 128],
                    in_=uac.broadcast_to((128, 128)),
                )

        # ---------------- phase 1: build xaT, xa_sum ----------------
        with ExitStack() as ctx:
            p1sb = ctx.enter_context(tc.tile_pool(name="p1sb", bufs=3))
            p1ps = ctx.enter_context(tc.tile_pool(name="p1ps", bufs=2, space="PSUM"))
            for m in range(NMAC):
                b0 = m * MB
                xt = p1sb.tile([128, MB, DOBJ], F32, tag="xt")
                nc.sync.dma_start(
                    out=xt,
                    in_=bass.AP(
                        tensor=x, offset=b0 * N * DOBJ,
                        ap=[[DOBJ, N], [N * DOBJ, MB], [1, DOBJ]],
                    ),
                )
                xm = p1sb.tile([128, MB, DOBJ], F32, tag="xm")
                nc.vector.tensor_tensor(
                    out=xm, in0=xt,
                    in1=maskT[:, b0:b0 + MB].unsqueeze(2).broadcast_to(
                        (128, MB, DOBJ)
                    ),
                    op=ALU.mult,
                )
                xa_ps = p1ps.tile([64, R], F32, tag="xa")
                for j in range(MB):
                    nc.tensor.transpose(
                        xa_ps[:, j * 128:(j + 1) * 128], xm[:, j, :], id_sb
                    )
                nc.scalar.copy(out=xaT[0:64, m * R:(m + 1) * R], in_=xa_ps)
                with nc.allow_low_precision(reason="f32r rounding of xa_sum"):
                    nc.vector.reduce_sum(
                        out=xasum[0:64, b0:b0 + MB],
                        in_=xa_ps.rearrange("p (j n) -> p j n", n=N),
                        axis=mybir.AxisListType.X,
                    )

        # ---------------- q_proj for all 256 batch ----------------
        with ExitStack() as ctx:
            qps = ctx.enter_context(tc.tile_pool(name="qps", bufs=2, space="PSUM"))
            qsb = ctx.enter_context(tc.tile_pool(name="qsb", bufs=2))
            for kc in range(KC):
                qp_ps = qps.tile([128, BSH], F32, tag="qp")
                nc.tensor.matmul(
                    qp_ps,
                    wauq[:, kc * 128:(kc + 1) * 128],
                    xasum,
                    start=True, stop=True,
                )
                qp_sb = qsb.tile([128, BSH], F32, tag="qpc")
                nc.vector.tensor_copy(out=qp_sb, in_=qp_ps)
                for h in range(2):
                    qpt_ps = qps.tile([128, 128], F32, tag="qpt")
                    nc.tensor.transpose(
                        qpt_ps, qp_sb[:, h * 128:(h + 1) * 128], id_sb
                    )
                    nc.vector.tensor_copy(
                        out=qptt[:, h * DM + kc * 128: h * DM + (kc + 1) * 128],
                        in_=qpt_ps,
                    )
            for h in range(2):
                nc.vector.tensor_scalar(
                    out=qptt[:, h * DM:(h + 1) * DM],
                    in0=qptt[:, h * DM:(h + 1) * DM],
                    scalar1=recipd[:, h:h + 1], scalar2=None, op0=ALU.mult,
                )

        # ---------------- phase 2: attention ----------------
        with ExitStack() as ctx:
            p2ps = ctx.enter_context(tc.tile_pool(name="p2ps", bufs=3, space="PSUM"))
            lps = ctx.enter_context(tc.tile_pool(name="lps", bufs=2, space="PSUM"))
            p2sb = ctx.enter_context(tc.tile_pool(name="p2sb", bufs=3))
            for m in range(NMAC):
                b0 = m * MB
                h = b0 // 128
                boff = b0 % 128
                par = m % 2
                # dynamic q_proj rows of the lhsT ring (one 4x512 SBUF->SBUF
                # DMA -- engine ops can't address partition 65, DMA can)
                nc.sync.dma_start(
                    out=rpw[65:69, par * DM:(par + 1) * DM],
                    in_=qptt[boff:boff + MB, h * DM:(h + 1) * DM],
                )
                zts = []
                for kc in range(KC):
                    rp_ps = p2ps.tile([128, R], F32, tag="rp")
                    nc.tensor.matmul(
                        rp_ps,
                        rpw[:, par * DM + kc * 128: par * DM + (kc + 1) * 128],
                        xaT[:, m * R:(m + 1) * R],
                        start=True, stop=True,
                    )
                    zt = p2sb.tile([128, R], DT_MM, tag="zt")
                    nc.scalar.activation(out=zt, in_=rp_ps, func=AF.Tanh)
                    zts.append(zt)
                logits_ps = lps.tile([128, R], F32, tag="lg")
                for kc in range(KC):
                    nc.tensor.matmul(
                        logits_ps,
                        uarep[:, kc * 128:(kc + 1) * 128],
                        zts[kc],
                        start=(kc == 0), stop=(kc == KC - 1),
                    )
                e_sb = p2sb.tile([65, R], F32, tag="e")
                nc.scalar.activation(out=e_sb, in_=logits_ps[0:65, :], func=AF.Exp)
                prod = p2sb.tile([65, R], F32, tag="prod")
                nc.vector.tensor_tensor(
                    out=prod, in0=xaT[0:65, m * R:(m + 1) * R].bitcast(F32), in1=e_sb,
                    op=ALU.mult,
                )
                nc.vector.reduce_sum(
                    out=xaesum[0:65, b0:b0 + MB],
                    in_=prod.rearrange("p (j n) -> p j n", n=N),
                    axis=mybir.AxisListType.X,
                )

        # ---------------- final: normalize + output ----------------
        with ExitStack() as ctx:
            fps = ctx.enter_context(tc.tile_pool(name="fps", bufs=2, space="PSUM"))
            fsb = ctx.enter_context(tc.tile_pool(name="fsb", bufs=2))
            for h in range(2):
                zc_ps = fps.tile([128, 1], F32, tag="zc")
                nc.tensor.transpose(
                    zc_ps, xaesum[64:65, h * 128:(h + 1) * 128],
                    id_sb[64:65, 64:65],
                )
                zc2 = fsb.tile([128, 1], F32, tag="zc2")
                nc.vector.tensor_scalar(
                    out=zc2, in0=zc_ps, scalar1=1e-30, scalar2=None, op0=ALU.add
                )
                nc.vector.reciprocal(out=recipz[:, h:h + 1], in_=zc2)
            for h in range(2):
                out_ps = fps.tile([128, DM], F32, tag="op")
                nc.tensor.matmul(
                    out_ps, xaesum[0:65, h * 128:(h + 1) * 128], wb,
                    start=True, stop=True,
                )
                out_sb = fsb.tile([128, DM], F32, tag="ob")
                nc.vector.tensor_scalar(
                    out=out_sb, in0=out_ps, scalar1=recipz[:, h:h + 1],
                    scalar2=None, op0=ALU.mult,
                )
                nc.sync.dma_start(
                    out=out.ap()[h * 128:(h + 1) * 128, :], in_=out_sb
                )

    nc.compile()
    return nc


def _ensure_ntff_hook():
    """Provide antenv.axon_hooks if the image lacks it (NTFF profiling via
    ctypes into libaxon_pjrt.so), and stub out the artifact upload."""
    import types
    import ctypes
    import contextlib

    try:
        from antenv.axon_hooks import get_axon_ntff_profile_hook  # noqa: F401
    except ImportError:
        so_path = "/opt/axon/libaxon_pjrt.so"
        hook = None
        if os.path.exists(so_path):
            lib = ctypes.CDLL(so_path)
            if hasattr(lib, "axon_start_nrt_profile"):
                lib.axon_start_nrt_profile.argtypes = [
                    ctypes.POINTER(ctypes.c_int64), ctypes.c_size_t,
                ]
                lib.axon_start_nrt_profile.restype = ctypes.c_int64
                lib.axon_stop_nrt_profile.argtypes = [ctypes.c_char_p]
                lib.axon_stop_nrt_profile.restype = ctypes.c_int64

                @contextlib.contextmanager
                def _hook(output_dir, device_ids):
                    import jax
                    jax.devices()
                    if device_ids:
                        ids = (ctypes.c_int64 * len(device_ids))(*device_ids)
                        rc = lib.axon_start_nrt_profile(ids, len(device_ids))
                    else:
                        rc = lib.axon_start_nrt_profile(None, 0)
                    if rc != 0:
                        raise RuntimeError(f"axon_start_nrt_profile rc={rc}")
                    try:
                        yield
                    finally:
                        n = lib.axon_stop_nrt_profile(str(output_dir).encode())
                        print(f"ntff profile: {n} file(s) -> {output_dir}",
                              file=sys.stderr)

                hook = _hook

        import antenv
        mod = types.ModuleType("antenv.axon_hooks")
        mod.get_axon_ntff_profile_hook = lambda: hook
        mod.set_axon_ntff_profile_hook = lambda h: None
        sys.modules["antenv.axon_hooks"] = mod
        antenv.axon_hooks = mod

    import concourse.bass_utils as bu
    bu.upload_artifacts = lambda tmpdir: f"file://{tmpdir}"


def make_in_maps(x_others, x_mask):
    """Shard batch across 8 cores; replicate weights; add constants."""
    ind4 = np.zeros((MB, R), dtype=np.float32)
    for j in range(MB):
        ind4[j, j * N:(j + 1) * N] = 1.0
    ident = np.eye(128, dtype=np.float32)
    return ind4, ident


def kernel(x_others, x_mask, conv_w, conv_b, Uq, Ur, Ua):
    x_others = np.asarray(x_others, dtype=np.float32)
    x_mask = np.asarray(x_mask, dtype=np.float32)
    conv_w = np.ascontiguousarray(np.asarray(conv_w, dtype=np.float32))
    conv_b = np.asarray(conv_b, dtype=np.float32).reshape(1, DM)
    Uq = np.ascontiguousarray(np.asarray(Uq, dtype=np.float32))
    Ur = np.ascontiguousarray(np.asarray(Ur, dtype=np.float32))
    Ua = np.asarray(Ua, dtype=np.float32).reshape(1, DM)
    ind4, ident = make_in_maps(x_others, x_mask)

    nc = build_nc()

    in_maps = []
    for c in range(NCORES):
        sl = slice(c * BSH, (c + 1) * BSH)
        in_maps.append({
            "x": np.ascontiguousarray(x_others[sl]),
            "mask": np.ascontiguousarray(x_mask[sl]),
            "conv_w": conv_w,
            "conv_b": conv_b,
            "Uq": Uq,
            "Ur": Ur,
            "Ua": Ua,
            "ind4": ind4,
            "ident": ident,
        })

    from concourse.bass_utils import run_bass_kernel_spmd

    trace = os.environ.get("KERNEL_TRACE", "0") == "1"
    if trace:
        _ensure_ntff_hook()
    tmpdir = None
    if trace:
        import tempfile
        os.makedirs("/root/problem/traces", exist_ok=True)
        tmpdir = tempfile.mkdtemp(dir="/root/problem/traces")
        print(f"trace dir: {tmpdir}", file=sys.stderr)
    res = run_bass_kernel_spmd(
        nc, in_maps, core_ids=list(range(NCORES)), trace=trace, tmpdir=tmpdir
    )
    if trace and res.exec_time_ns is not None:
        print(f"HW exec time: {res.exec_time_ns} ns", file=sys.stderr)
        kernel.last_exec_time_ns = res.exec_time_ns
        kernel.last_trace = res.instructions_and_trace
    out = np.concatenate([r["out"] for r in res.results], axis=0)
    return out


if __name__ == "__main__":
    rng = np.random.default_rng(0)
    x = rng.standard_normal((B, N * DOBJ), dtype=np.float32)
    mask = rng.integers(0, 2, (B, N)).astype(np.float32)
    w = rng.standard_normal((DOBJ, DM), dtype=np.float32) / 8.0
    cbv = np.zeros((DM,), dtype=np.float32)
    uq = rng.standard_normal((DM, DM), dtype=np.float32) / 22.6
    urm = rng.standard_normal((DM, DM), dtype=np.float32) / 22.6
    uav = rng.standard_normal((DM,), dtype=np.float32) * 0.1
    out = kernel(x, mask, w, cbv, uq, urm, uav)
    print(out.shape, out.dtype)
